# revision 2
# baseline (speedup 1.0000x reference)
"""Multi-head attention (16 heads, d_model=1024, head_dim=64) on 8 trn2 cores.

Sharding: core c handles batch b = c//2 and heads [8*(c%2), 8*(c%2)+8)
(data parallel over batch x tensor parallel over heads). Each core
computes its 8 heads' Q/K/V projections, attention, and a partial output
projection; the host sums the two partial projections per batch element
(the "all-reduce") and adds the output bias.

Device-side layout is feature-major ("transposed"): projections produce
Q^T/K^T [d, t] so that the attention matmuls contract along partitions.

The two heads of a head-pair are processed in ONE pair-unit: their
energy matmuls are emitted back-to-back as 64x128 row tiles at
positions (0,0)/(64,0) with a dedicated top-priority band, so the PE
runs the pair concurrently on disjoint row groups (per-subarray
concurrency + LDWEIGHTS pull-ahead) and the K=64 energies recover the
idle half of the array. One N=1024 exp covers both heads per key
chunk. All other PE work (attn@V chains, output projection, next
pair's Q/K tiles) is flattened into generators and pumped ~4 matmuls
per exp slot so the scalar engine streams activations back-to-back.
Softmax normalization runs off the PE entirely: row-sum reciprocals on
the DVE, the partition broadcast on the (otherwise idle) GPSIMD, and
the scale on the DVE. V bias is applied on the DVE against a broadcast
tile. Output staging and the out DRAM tensor are bf16 (host sums the
two partial projections per batch element in fp32).

All matmul inputs are bf16 (fp32 PSUM accumulation); softmax is
unnormalized exp (energies bounded ~|15|) with the row-sum from an
extra ones-column in the attn@V matmul.
"""

import numpy as np
import ml_dtypes

from concourse import bass, bacc, tile, mybir
from concourse.tile_rust import add_dep_helper
from concourse.bass_utils import run_bass_kernel_spmd

BF16 = ml_dtypes.bfloat16
dt = mybir.dt
AF = mybir.ActivationFunctionType

N_CORES = 8
T = 2048          # tokens per batch element
D = 1024          # model dim
FH = 512          # features (head dims) per core: 8 heads x 64
NH_LOC = 8        # heads per core
HD = 64           # head dim

_prog_cache = {}


def _build_program():
    nc = bacc.Bacc("TRN2", target_bir_lowering=False, debug=False,
                   num_devices=N_CORES)

    xT = nc.dram_tensor("xT", [D, T], dt.bfloat16, kind="ExternalInput").ap()
    wqT = nc.dram_tensor("wqT", [D, FH], dt.bfloat16, kind="ExternalInput").ap()
    wkT = nc.dram_tensor("wkT", [D, FH], dt.bfloat16, kind="ExternalInput").ap()
    wvT = nc.dram_tensor("wvT", [D, FH], dt.bfloat16, kind="ExternalInput").ap()
    bqT = nc.dram_tensor("bqT", [128, 4], dt.float32, kind="ExternalInput").ap()
    bkT = nc.dram_tensor("bkT", [128, 4], dt.float32, kind="ExternalInput").ap()
    bvs = nc.dram_tensor("bvs", [1, FH], dt.bfloat16, kind="ExternalInput").ap()
    wpT = nc.dram_tensor("wpT", [FH, D], dt.bfloat16, kind="ExternalInput").ap()
    ones = nc.dram_tensor("ones", [1, 128], dt.bfloat16, kind="ExternalInput").ap()
    out = nc.dram_tensor("out", [T, D], dt.bfloat16, kind="ExternalOutput").ap()

    with tile.TileContext(nc) as tc:
        _emit(tc, out, xT, wqT, wkT, wvT, bqT, bkT, bvs, wpT, ones)
    nc.compile()
    return nc


def _emit(tc, out, xT, wqT, wkT, wvT, bqT, bkT, bvs, wpT, ones):
    nc = tc.nc
    f32 = dt.float32
    bf16 = dt.bfloat16

    with (
        tc.tile_pool(name="sbp", bufs=1) as sbp,
        tc.tile_pool(name="qkv_sb", bufs=1) as qkv_sb,
        tc.tile_pool(name="pb_pool", bufs=2) as pb_pool,
        tc.tile_pool(name="rr_pool", bufs=2) as rr_pool,
        tc.tile_pool(name="bc_pool", bufs=2) as bc_pool,
        tc.tile_pool(name="ostage", bufs=2) as ostage,
        # PSUM: 4 banks for energies (2-bank tiles, ping-pong), 2 for
        # attn@V accumulators, 2 shared by V / Q,K tiles / output proj.
        tc.tile_pool(name="ps_e", bufs=2, space="PSUM") as ps_e,
        tc.tile_pool(name="ps_av", bufs=2, space="PSUM") as ps_av,
        tc.tile_pool(name="ps_misc", bufs=2, space="PSUM") as ps_misc,
    ):
        # Input DMAs split across the two HW-DGE queues (SP + ACT), in
        # first-use order: K weights and x feed the first matmuls.
        ones_s = sbp.tile([1, 128], bf16)
        nc.sync.dma_start(out=ones_s[:], in_=ones)
        bkT_s = sbp.tile([128, 4], f32)
        nc.sync.dma_start(out=bkT_s[:], in_=bkT)
        bqT_s = sbp.tile([128, 4], f32)
        nc.sync.dma_start(out=bqT_s[:], in_=bqT)

        wk_s = sbp.tile([128, 8, FH], bf16, tag="wk")
        nc.sync.dma_start(out=wk_s[:], in_=wkT.rearrange("(m p) d -> p m d", p=128))
        x_s = sbp.tile([128, 8, T], bf16)
        xr = xT.rearrange("(m p) t -> p m t", p=128)
        for m in range(8):
            eng = nc.sync if m % 2 == 0 else nc.scalar
            eng.dma_start(out=x_s[:, m, :], in_=xr[:, m, :])
        wq_s = sbp.tile([128, 8, FH], bf16, tag="wq")
        nc.scalar.dma_start(out=wq_s[:], in_=wqT.rearrange("(m p) d -> p m d", p=128))
        bvs_s = sbp.tile([1, FH], bf16)
        nc.sync.dma_start(out=bvs_s[:], in_=bvs)
        wv_s = sbp.tile([128, 8, FH], bf16, tag="wv")
        nc.scalar.dma_start(out=wv_s[:], in_=wvT.rearrange("(m p) d -> p m d", p=128))
        wp_s = sbp.tile([128, 4, D], bf16)
        nc.sync.dma_start(out=wp_s[:], in_=wpT.rearrange("(c p) o -> p c o", p=128))

        # QT/KT: [d-in-pair(128), head-pair(4), t]; V: [t-in-chunk(128),
        # t-chunk(16), head(8), 65] with col 64 = 1.0 (row-sum trick).
        # Q^T/K^T live only while their pair streams (+1 pair prefill):
        # 2-slot rings indexed hp % 2.
        QT_sb = qkv_sb.tile([128, 2, T], bf16)
        KT_sb = qkv_sb.tile([128, 2, T], bf16)
        V_sb = qkv_sb.tile([128, 16, NH_LOC, 65], bf16)
        nc.vector.memset(V_sb[:, :, :, 64:65], 1.0)
        bvb_s = qkv_sb.tile([128, NH_LOC, 64], bf16)
        # AttnOut^T: [f-in-chunk(128), f-chunk(4), t]
        AO_sb = qkv_sb.tile([128, 4, T], bf16)

        def emit_qk_ntile(w_s, b_s, dst, hp, n, anchor=None):
            # one n-tile of a Q^T/K^T projection: an 8-matmul chain
            dsl = slice(hp * 128, (hp + 1) * 128)
            ps = ps_misc.tile([128, 512], f32, tag="m", name="qk_ps")
            for m in range(8):
                mm = nc.tensor.matmul(ps[:], w_s[:, m, dsl],
                                      x_s[:, m, n * 512:(n + 1) * 512],
                                      start=(m == 0), stop=(m == 7))
                if m == 0 and anchor is not None:
                    add_dep_helper(mm.ins, anchor.ins, sync=False,
                                   reason="filler pacing")
            nc.vector.tensor_scalar_add(
                dst[:, hp % 2, n * 512:(n + 1) * 512], ps[:],
                b_s[:, hp:hp + 1])

        def emit_bvb():
            # broadcast bv across the 128 partitions via a K=1 matmul
            ps = ps_misc.tile([128, 512], f32, tag="m", name="bvb_ps")
            nc.tensor.matmul(ps[:], ones_s[0:1, 0:128], bvs_s[:],
                             start=True, stop=True)
            nc.vector.tensor_copy(bvb_s[:], ps[:].rearrange(
                "p (h d) -> p h d", h=NH_LOC))

        def emit_v_tile(t, anchor=None):
            # V (natural): out[t, d] = x[t, :].wvT[:, d]; bias on the DVE
            ps = ps_misc.tile([128, 512], f32, tag="m", name="v_ps")
            for m in range(8):
                mm = nc.tensor.matmul(ps[:], x_s[:, m, t * 128:(t + 1) * 128],
                                      wv_s[:, m, :],
                                      start=(m == 0), stop=(m == 7))
                if m == 0 and anchor is not None:
                    add_dep_helper(mm.ins, anchor.ins, sync=False,
                                   reason="filler pacing")
            nc.vector.tensor_add(
                V_sb[:, t, :, 0:64],
                ps[:].rearrange("p (h d) -> p h d", h=NH_LOC), bvb_s[:])

        def emit_proj(t, anchor=None):
            # partial output projection (pre-bias) for token tile t
            tsl = slice(t * 128, (t + 1) * 128)
            st = ostage.tile([128, D], bf16, tag="st")
            ps0 = ps_misc.tile([128, 512], f32, tag="m", name="pj0")
            ps1 = ps_misc.tile([128, 512], f32, tag="m", name="pj1")
            for fc in range(4):
                mm = nc.tensor.matmul(ps0[:], AO_sb[:, fc, tsl],
                                      wp_s[:, fc, 0:512],
                                      start=(fc == 0), stop=(fc == 3))
                if fc == 0 and anchor is not None:
                    add_dep_helper(mm.ins, anchor.ins, sync=False,
                                   reason="filler pacing")
                nc.tensor.matmul(ps1[:], AO_sb[:, fc, tsl],
                                 wp_s[:, fc, 512:1024],
                                 start=(fc == 0), stop=(fc == 3))
            nc.vector.tensor_copy(st[:, 0:512], ps0[:])
            nc.vector.tensor_copy(st[:, 512:1024], ps1[:])
            nc.sync.dma_start(out=out[tsl, :], in_=st[:])

        # ---- software-pipelined attention over 16 pair-units ----
        units = [(hp, j) for hp in range(4) for j in range(4)]
        state = {}
        fillers = []

        def emit_e_slot(u, kc):
            # energies for BOTH heads of the pair, one key chunk: two
            # 64x128 row tiles at (0,0)/(64,0) run concurrently on the
            # PE; one N=1024 exp covers both heads.
            hp, j = u
            qsl = slice(j * 512, (j + 1) * 512)
            ksl = slice(kc * 128, (kc + 1) * 128)
            pb = state[u]["pb"]
            e2 = ps_e.tile([128, 2, 512], f32, tag="e")
            hs = hp % 2
            mmA = nc.tensor.matmul(e2[:, 0, :], KT_sb[0:64, hs, ksl],
                                   QT_sb[0:64, hs, qsl], start=True, stop=True)
            mmB = nc.tensor.matmul(e2[:, 1, :], KT_sb[64:128, hs, ksl],
                                   QT_sb[64:128, hs, qsl], start=True, stop=True)
            add_dep_helper(mmB.ins, mmA.ins, sync=False, reason="pair glue")
            mmA.ins.bass_priority = eprio[0]
            mmB.ins.bass_priority = eprio[0] + 1
            eprio[0] += 2
            return nc.scalar.activation(pb[:, kc, :, :], e2[:], AF.Exp)

        def emit_av_block(u, s, kcs, anchor=None):
            # attn@V accumulation matmuls (V col 64 is ones -> row sums)
            hp, j = u
            st = state[u]
            av = st["av"].get(s)
            if av is None:
                av = ps_av.tile([128, 512], f32, tag="av")
                st["av"][s] = av
            pb = st["pb"]
            first = True
            for kc in kcs:
                mm = nc.tensor.matmul(av[0:65, :], V_sb[:, kc, 2 * hp + s, 0:65],
                                      pb[:, kc, s, :],
                                      start=(kc == 0), stop=(kc == 15))
                if first and anchor is not None:
                    add_dep_helper(mm.ins, anchor.ins, sync=False,
                                   reason="filler pacing")
                first = False

        def emit_norm_a(u, s):
            # softmax normalization part 1 (DVE only): spill accumulator
            # rows to SBUF, reciprocal of the row sums. Frees the av bank.
            st = state[u]
            av = st["av"][s]
            avd = bc_pool.tile([64, 512], bf16, tag="avd", bufs=3)
            nc.vector.tensor_copy(avd[:], av[0:64, :])
            rr = rr_pool.tile([1, 512], f32, tag="rr", bufs=2)
            nc.vector.reciprocal(rr[:], av[64:65, :])
            rrb = rr_pool.tile([1, 512], bf16, tag="rrb", bufs=4)
            nc.vector.tensor_copy(rrb[:], rr[:])
            st["av"][s] = None
            st["avd"][s] = avd
            st["rrb"][s] = rrb

        def emit_norm_b(u, s, anchor=None):
            # part 2: broadcast 1/rowsum across the 64 head-dim partitions
            # via a K=1 matmul, then scale into AttnOut^T
            hp, j = u
            psl = slice(64 * s, 64 * s + 64)
            qsl = slice(j * 512, (j + 1) * 512)
            st = state[u]
            avd, rrb = st["avd"][s], st["rrb"][s]
            bcp = ps_misc.tile([64, 512], f32, tag="m", name="bcp")
            mm = nc.tensor.matmul(bcp[:], ones_s[0:1, 0:64], rrb[:],
                                  start=True, stop=True)
            if anchor is not None:
                add_dep_helper(mm.ins, anchor.ins, sync=False,
                               reason="filler pacing")
            nc.vector.tensor_mul(AO_sb[psl, hp, qsl], avd[:], bcp[:])

        def pop_filler(anchor=None):
            if fillers:
                fillers.pop(0)(anchor)

        # prologue: K projection (all key tiles) + Q tile 0 of pair 0;
        # Q tiles 1-3 ride inside/behind unit 0.
        for n in range(4):
            emit_qk_ntile(wk_s, bkT_s, KT_sb, 0, n)
        emit_qk_ntile(wq_s, bqT_s, QT_sb, 0, 0)
        emit_bvb()
        fillers.append(lambda a: emit_qk_ntile(wq_s, bqT_s, QT_sb, 0, 2, anchor=a))
        fillers.append(lambda a: emit_qk_ntile(wq_s, bqT_s, QT_sb, 0, 3, anchor=a))

        P = None   # previous pair-unit (attn@V source)
        P2 = None  # the one before (norm_b)
        for ui, u in enumerate(units):
            hp, j = u
            if j == 0 and hp < 3:
                for w_s, b_s, dst in ((wk_s, bkT_s, KT_sb),
                                      (wq_s, bqT_s, QT_sb)):
                    for n in range(4):
                        fillers.append(
                            lambda a, w=w_s, b=b_s, d=dst, p=hp + 1, nn=n:
                            emit_qk_ntile(w, b, d, p, nn, anchor=a))
            state[u] = {"pb": pb_pool.tile([128, 16, 2, 512], bf16, tag="pb",
                                           name="pb"),
                        "av": {}, "avd": {}, "rrb": {}}
            if ui >= 2:
                pop_filler()
            for kc in range(16):
                act = emit_e_slot(u, kc)
                if ui == 0:
                    if kc == 5:
                        emit_qk_ntile(wq_s, bqT_s, QT_sb, 0, 1, anchor=act)
                    if kc % 2 == 1:
                        emit_v_tile(kc // 2, anchor=act)
                elif ui == 1 and kc < 8:
                    emit_v_tile(8 + kc, anchor=act)
                if P is not None:
                    if kc == 0:
                        emit_av_block(P, 0, range(0, 8), anchor=act)
                    elif kc == 4:
                        emit_av_block(P, 1, range(0, 8), anchor=act)
                    elif kc == 8:
                        emit_av_block(P, 0, range(8, 16), anchor=act)
                    elif kc == 10:
                        emit_norm_a(P, 0)
                    elif kc == 12:
                        emit_av_block(P, 1, range(8, 16), anchor=act)
                    elif kc == 15:
                        emit_norm_a(P, 1)
                if P2 is not None:
                    if kc == 2:
                        emit_norm_b(P2, 0, anchor=act)
                    elif kc == 6:
                        emit_norm_b(P2, 1, anchor=act)
                        if P2[0] == 3:
                            for tt in range(4):
                                fillers.append(
                                    lambda a, t=4 * P2[1] + tt:
                                    emit_proj(t, anchor=a))
                if ui >= 2 and kc in (1, 3, 5, 7, 9, 11, 14):
                    pop_filler(act)
            P2 = P
            P = u

        # pipeline tail
        emit_av_block(P, 0, range(0, 8))
        emit_av_block(P, 0, range(8, 16))
        emit_norm_a(P, 0)
        emit_av_block(P, 1, range(0, 8))
        emit_av_block(P, 1, range(8, 16))
        emit_norm_a(P, 1)
        emit_norm_b(P2, 0)
        emit_norm_b(P2, 1)
        for tt in range(4):
            fillers.append(lambda a, t=8 + tt: emit_proj(t, anchor=a))
        pop_filler()
        pop_filler()
        emit_norm_b(P, 0)
        emit_norm_b(P, 1)
        for tt in range(4):
            fillers.append(lambda a, t=12 + tt: emit_proj(t, anchor=a))
        while fillers:
            pop_filler()


def get_program():
    if "nc" not in _prog_cache:
        _prog_cache["nc"] = _build_program()
    return _prog_cache["nc"]


def make_in_maps(inputs):
    x = np.asarray(inputs["x"], dtype=np.float32)
    Wq = np.asarray(inputs["Wq"], dtype=np.float32)
    bq = np.asarray(inputs["bq"], dtype=np.float32)
    Wk = np.asarray(inputs["Wk"], dtype=np.float32)
    bk = np.asarray(inputs["bk"], dtype=np.float32)
    Wv = np.asarray(inputs["Wv"], dtype=np.float32)
    bv = np.asarray(inputs["bv"], dtype=np.float32)
    Wp = np.asarray(inputs["Wp"], dtype=np.float32)

    ones_h = np.ones((1, 128), dtype=BF16)
    in_maps = []
    for c in range(N_CORES):
        b, half = divmod(c, 2)
        fs = slice(half * FH, half * FH + FH)
        in_maps.append({
            "xT": np.ascontiguousarray(x[b].T).astype(BF16),
            "wqT": np.ascontiguousarray(Wq[fs].T).astype(BF16),
            "wkT": np.ascontiguousarray(Wk[fs].T).astype(BF16),
            "wvT": np.ascontiguousarray(Wv[fs].T).astype(BF16),
            "bqT": np.ascontiguousarray(bq[fs].reshape(4, 128).T),
            "bkT": np.ascontiguousarray(bk[fs].reshape(4, 128).T),
            "bvs": bv[fs].astype(BF16).reshape(1, FH),
            "wpT": np.ascontiguousarray(Wp[:, fs].T).astype(BF16),
            "ones": ones_h,
        })
    return in_maps


def gather_output(results, bp):
    bp = np.asarray(bp, dtype=np.float32)
    return np.stack([
        results[2 * b]["out"].astype(np.float32)
        + results[2 * b + 1]["out"].astype(np.float32) + bp[None, :]
        for b in range(4)
    ]).astype(np.float32)


def kernel(**inputs):
    nc = get_program()
    in_maps = make_in_maps(inputs)
    res = run_bass_kernel_spmd(nc, in_maps, list(range(N_CORES))).results
    return gather_output(res, inputs["bp"])


# revision 3
# speedup vs baseline: 6522.4102x; 6522.4102x over previous
"""Multi-head attention (16 heads, d_model=1024, head_dim=64) on 8 trn2 cores.

Sharding: core c handles batch b = c//2 and heads [8*(c%2), 8*(c%2)+8)
(data parallel over batch x tensor parallel over heads). Each core
computes its 8 heads' Q/K/V projections, attention, and a partial output
projection; the host sums the two partial projections per batch element
(the "all-reduce") and adds the output bias.

Device-side layout is feature-major ("transposed"): projections produce
Q^T/K^T [d, t] so that the attention matmuls contract along partitions.

The two heads of a head-pair are processed in ONE pair-unit: their
energy matmuls are emitted back-to-back as 64x128 row tiles at
positions (0,0)/(64,0) with a dedicated top-priority band, so the PE
runs the pair concurrently on disjoint row groups (per-subarray
concurrency + LDWEIGHTS pull-ahead) and the K=64 energies recover the
idle half of the array. One N=1024 exp covers both heads per key
chunk. All other PE work (attn@V chains, output projection, next
pair's Q/K tiles) is flattened into generators and pumped ~4 matmuls
per exp slot so the scalar engine streams activations back-to-back.
Softmax normalization runs off the PE entirely: row-sum reciprocals on
the DVE, the partition broadcast on the (otherwise idle) GPSIMD, and
the scale on the DVE. V bias is applied on the DVE against a broadcast
tile, and the V projection is split by head halves: heads 0-3 up
front, heads 4-7 (first consumed by pair 2's attention) spread over
mid-stream units. Output staging and the out DRAM tensor are bf16
(host sums the two partial projections per batch element in fp32).

All matmul inputs are bf16 (fp32 PSUM accumulation); softmax is
unnormalized exp (energies bounded ~|15|) with the row-sum from an
extra ones-column in the attn@V matmul.
"""

import numpy as np
import ml_dtypes

from concourse import bass, bacc, tile, mybir
from concourse.tile_rust import add_dep_helper
from concourse.bass_utils import run_bass_kernel_spmd

BF16 = ml_dtypes.bfloat16
dt = mybir.dt
AF = mybir.ActivationFunctionType

N_CORES = 8
T = 2048          # tokens per batch element
D = 1024          # model dim
FH = 512          # features (head dims) per core: 8 heads x 64
NH_LOC = 8        # heads per core
HD = 64           # head dim

_prog_cache = {}


def _build_program():
    nc = bacc.Bacc("TRN2", target_bir_lowering=False, debug=False,
                   num_devices=N_CORES)

    xT = nc.dram_tensor("xT", [D, T], dt.bfloat16, kind="ExternalInput").ap()
    wqT = nc.dram_tensor("wqT", [D, FH], dt.bfloat16, kind="ExternalInput").ap()
    wkT = nc.dram_tensor("wkT", [D, FH], dt.bfloat16, kind="ExternalInput").ap()
    wvT = nc.dram_tensor("wvT", [D, FH], dt.bfloat16, kind="ExternalInput").ap()
    bqT = nc.dram_tensor("bqT", [128, 4], dt.float32, kind="ExternalInput").ap()
    bkT = nc.dram_tensor("bkT", [128, 4], dt.float32, kind="ExternalInput").ap()
    bvs = nc.dram_tensor("bvs", [1, FH], dt.bfloat16, kind="ExternalInput").ap()
    wpT = nc.dram_tensor("wpT", [FH, D], dt.bfloat16, kind="ExternalInput").ap()
    ones = nc.dram_tensor("ones", [1, 128], dt.bfloat16, kind="ExternalInput").ap()
    out = nc.dram_tensor("out", [T, D], dt.bfloat16, kind="ExternalOutput").ap()

    with tile.TileContext(nc) as tc:
        _emit(tc, out, xT, wqT, wkT, wvT, bqT, bkT, bvs, wpT, ones)
    nc.compile()
    return nc


def _emit(tc, out, xT, wqT, wkT, wvT, bqT, bkT, bvs, wpT, ones):
    nc = tc.nc
    f32 = dt.float32
    bf16 = dt.bfloat16

    with (
        tc.tile_pool(name="sbp", bufs=1) as sbp,
        tc.tile_pool(name="qkv_sb", bufs=1) as qkv_sb,
        tc.tile_pool(name="pb_pool", bufs=2) as pb_pool,
        tc.tile_pool(name="rr_pool", bufs=2) as rr_pool,
        tc.tile_pool(name="bc_pool", bufs=2) as bc_pool,
        tc.tile_pool(name="ostage", bufs=2) as ostage,
        # PSUM: 4 banks for energies (2-bank tiles, ping-pong), 2 for
        # attn@V accumulators, 2 shared by V / Q,K tiles / output proj.
        tc.tile_pool(name="ps_e", bufs=2, space="PSUM") as ps_e,
        tc.tile_pool(name="ps_av", bufs=2, space="PSUM") as ps_av,
        tc.tile_pool(name="ps_misc", bufs=2, space="PSUM") as ps_misc,
    ):
        # Input DMAs split across the two HW-DGE queues (SP + ACT), in
        # first-use order: K weights and x feed the first matmuls.
        ones_s = sbp.tile([1, 128], bf16)
        nc.sync.dma_start(out=ones_s[:], in_=ones)
        bkT_s = sbp.tile([128, 4], f32)
        nc.sync.dma_start(out=bkT_s[:], in_=bkT)
        bqT_s = sbp.tile([128, 4], f32)
        nc.sync.dma_start(out=bqT_s[:], in_=bqT)

        wk_s = sbp.tile([128, 8, FH], bf16, tag="wk")
        nc.sync.dma_start(out=wk_s[:], in_=wkT.rearrange("(m p) d -> p m d", p=128))
        x_s = sbp.tile([128, 8, T], bf16)
        xr = xT.rearrange("(m p) t -> p m t", p=128)
        for m in range(8):
            eng = nc.sync if m % 2 == 0 else nc.scalar
            eng.dma_start(out=x_s[:, m, :], in_=xr[:, m, :])
        wq_s = sbp.tile([128, 8, FH], bf16, tag="wq")
        nc.scalar.dma_start(out=wq_s[:], in_=wqT.rearrange("(m p) d -> p m d", p=128))
        bvs_s = sbp.tile([1, FH], bf16)
        nc.sync.dma_start(out=bvs_s[:], in_=bvs)
        wv_s = sbp.tile([128, 8, FH], bf16, tag="wv")
        nc.scalar.dma_start(out=wv_s[:], in_=wvT.rearrange("(m p) d -> p m d", p=128))
        wp_s = sbp.tile([128, 4, D], bf16)
        nc.sync.dma_start(out=wp_s[:], in_=wpT.rearrange("(c p) o -> p c o", p=128))

        # QT/KT: [d-in-pair(128), head-pair(4), t]; V: [t-in-chunk(128),
        # t-chunk(16), head(8), 65] with col 64 = 1.0 (row-sum trick).
        # Q^T/K^T live only while their pair streams (+1 pair prefill):
        # 2-slot rings indexed hp % 2.
        QT_sb = qkv_sb.tile([128, 2, T], bf16)
        KT_sb = qkv_sb.tile([128, 2, T], bf16)
        V_sb = qkv_sb.tile([128, 16, NH_LOC, 65], bf16)
        nc.vector.memset(V_sb[:, :, :, 64:65], 1.0)
        bvb_s = qkv_sb.tile([128, NH_LOC, 64], bf16)
        # AttnOut^T: [f-in-chunk(128), f-chunk(4), t]
        AO_sb = qkv_sb.tile([128, 4, T], bf16)

        def emit_qk_ntile(w_s, b_s, dst, hp, n, anchor=None):
            # one n-tile of a Q^T/K^T projection: an 8-matmul chain
            dsl = slice(hp * 128, (hp + 1) * 128)
            ps = ps_misc.tile([128, 512], f32, tag="m", name="qk_ps")
            for m in range(8):
                mm = nc.tensor.matmul(ps[:], w_s[:, m, dsl],
                                      x_s[:, m, n * 512:(n + 1) * 512],
                                      start=(m == 0), stop=(m == 7))
                if m == 0 and anchor is not None:
                    add_dep_helper(mm.ins, anchor.ins, sync=False,
                                   reason="filler pacing")
            nc.vector.tensor_scalar_add(
                dst[:, hp % 2, n * 512:(n + 1) * 512], ps[:],
                b_s[:, hp:hp + 1])

        def emit_bvb():
            # broadcast bv across the 128 partitions via a K=1 matmul
            ps = ps_misc.tile([128, 512], f32, tag="m", name="bvb_ps")
            nc.tensor.matmul(ps[:], ones_s[0:1, 0:128], bvs_s[:],
                             start=True, stop=True)
            nc.vector.tensor_copy(bvb_s[:], ps[:].rearrange(
                "p (h d) -> p h d", h=NH_LOC))

        def emit_v_tile(t, anchor=None):
            # V (natural): out[t, d] = x[t, :].wvT[:, d]; bias on the DVE
            ps = ps_misc.tile([128, 512], f32, tag="m", name="v_ps")
            for m in range(8):
                mm = nc.tensor.matmul(ps[:], x_s[:, m, t * 128:(t + 1) * 128],
                                      wv_s[:, m, :],
                                      start=(m == 0), stop=(m == 7))
                if m == 0 and anchor is not None:
                    add_dep_helper(mm.ins, anchor.ins, sync=False,
                                   reason="filler pacing")
            nc.vector.tensor_add(
                V_sb[:, t, :, 0:64],
                ps[:].rearrange("p (h d) -> p h d", h=NH_LOC), bvb_s[:])

        def emit_proj(t, anchor=None):
            # partial output projection (pre-bias) for token tile t
            tsl = slice(t * 128, (t + 1) * 128)
            st = ostage.tile([128, D], bf16, tag="st")
            ps0 = ps_misc.tile([128, 512], f32, tag="m", name="pj0")
            ps1 = ps_misc.tile([128, 512], f32, tag="m", name="pj1")
            for fc in range(4):
                mm = nc.tensor.matmul(ps0[:], AO_sb[:, fc, tsl],
                                      wp_s[:, fc, 0:512],
                                      start=(fc == 0), stop=(fc == 3))
                if fc == 0 and anchor is not None:
                    add_dep_helper(mm.ins, anchor.ins, sync=False,
                                   reason="filler pacing")
                nc.tensor.matmul(ps1[:], AO_sb[:, fc, tsl],
                                 wp_s[:, fc, 512:1024],
                                 start=(fc == 0), stop=(fc == 3))
            nc.vector.tensor_copy(st[:, 0:512], ps0[:])
            nc.vector.tensor_copy(st[:, 512:1024], ps1[:])
            nc.sync.dma_start(out=out[tsl, :], in_=st[:])

        # ---- software-pipelined attention over 16 pair-units ----
        units = [(hp, j) for hp in range(4) for j in range(4)]
        state = {}
        fillers = []

        def emit_e_slot(u, kc):
            # energies for BOTH heads of the pair, one key chunk: two
            # 64x128 row tiles at (0,0)/(64,0) run concurrently on the
            # PE; one N=1024 exp covers both heads.
            hp, j = u
            qsl = slice(j * 512, (j + 1) * 512)
            ksl = slice(kc * 128, (kc + 1) * 128)
            pb = state[u]["pb"]
            e2 = ps_e.tile([128, 2, 512], f32, tag="e")
            hs = hp % 2
            mmA = nc.tensor.matmul(e2[:, 0, :], KT_sb[0:64, hs, ksl],
                                   QT_sb[0:64, hs, qsl], start=True, stop=True)
            mmB = nc.tensor.matmul(e2[:, 1, :], KT_sb[64:128, hs, ksl],
                                   QT_sb[64:128, hs, qsl], start=True, stop=True)
            add_dep_helper(mmB.ins, mmA.ins, sync=False, reason="pair glue")
            mmA.ins.bass_priority = eprio[0]
            mmB.ins.bass_priority = eprio[0] + 1
            eprio[0] += 2
            return nc.scalar.activation(pb[:, kc, :, :], e2[:], AF.Exp)

        def emit_av_block(u, s, kcs, anchor=None):
            # attn@V accumulation matmuls (V col 64 is ones -> row sums)
            hp, j = u
            st = state[u]
            av = st["av"].get(s)
            if av is None:
                av = ps_av.tile([128, 512], f32, tag="av")
                st["av"][s] = av
            pb = st["pb"]
            first = True
            for kc in kcs:
                mm = nc.tensor.matmul(av[0:65, :], V_sb[:, kc, 2 * hp + s, 0:65],
                                      pb[:, kc, s, :],
                                      start=(kc == 0), stop=(kc == 15))
                if first and anchor is not None:
                    add_dep_helper(mm.ins, anchor.ins, sync=False,
                                   reason="filler pacing")
                first = False

        def emit_norm_a(u, s):
            # softmax normalization part 1 (DVE only): spill accumulator
            # rows to SBUF, reciprocal of the row sums. Frees the av bank.
            st = state[u]
            av = st["av"][s]
            avd = bc_pool.tile([64, 512], bf16, tag="avd", bufs=3)
            nc.vector.tensor_copy(avd[:], av[0:64, :])
            rr = rr_pool.tile([1, 512], f32, tag="rr", bufs=2)
            nc.vector.reciprocal(rr[:], av[64:65, :])
            rrb = rr_pool.tile([1, 512], bf16, tag="rrb", bufs=4)
            nc.vector.tensor_copy(rrb[:], rr[:])
            st["av"][s] = None
            st["avd"][s] = avd
            st["rrb"][s] = rrb

        def emit_norm_b(u, s, anchor=None):
            # part 2: broadcast 1/rowsum across the 64 head-dim partitions
            # via a K=1 matmul, then scale into AttnOut^T
            hp, j = u
            psl = slice(64 * s, 64 * s + 64)
            qsl = slice(j * 512, (j + 1) * 512)
            st = state[u]
            avd, rrb = st["avd"][s], st["rrb"][s]
            bcp = ps_misc.tile([64, 512], f32, tag="m", name="bcp")
            mm = nc.tensor.matmul(bcp[:], ones_s[0:1, 0:64], rrb[:],
                                  start=True, stop=True)
            if anchor is not None:
                add_dep_helper(mm.ins, anchor.ins, sync=False,
                               reason="filler pacing")
            nc.vector.tensor_mul(AO_sb[psl, hp, qsl], avd[:], bcp[:])

        def pop_filler(anchor=None):
            if fillers:
                fillers.pop(0)(anchor)

        # prologue: K projection (all key tiles) + Q tile 0 of pair 0;
        # Q tiles 1-3 ride inside/behind unit 0.
        for n in range(4):
            emit_qk_ntile(wk_s, bkT_s, KT_sb, 0, n)
        emit_qk_ntile(wq_s, bqT_s, QT_sb, 0, 0)
        emit_bvb()
        fillers.append(lambda a: emit_qk_ntile(wq_s, bqT_s, QT_sb, 0, 2, anchor=a))
        fillers.append(lambda a: emit_qk_ntile(wq_s, bqT_s, QT_sb, 0, 3, anchor=a))

        P = None   # previous pair-unit (attn@V source)
        P2 = None  # the one before (norm_b)
        for ui, u in enumerate(units):
            hp, j = u
            if j == 0 and hp < 3:
                for w_s, b_s, dst in ((wk_s, bkT_s, KT_sb),
                                      (wq_s, bqT_s, QT_sb)):
                    for n in range(4):
                        fillers.append(
                            lambda a, w=w_s, b=b_s, d=dst, p=hp + 1, nn=n:
                            emit_qk_ntile(w, b, d, p, nn, anchor=a))
            state[u] = {"pb": pb_pool.tile([128, 16, 2, 512], bf16, tag="pb",
                                           name="pb"),
                        "av": {}, "avd": {}, "rrb": {}}
            if ui >= 2:
                pop_filler()
            for kc in range(16):
                act = emit_e_slot(u, kc)
                if ui == 0:
                    if kc == 5:
                        emit_qk_ntile(wq_s, bqT_s, QT_sb, 0, 1, anchor=act)
                    if kc % 2 == 1:
                        emit_v_tile(kc // 2, anchor=act)
                elif ui == 1 and kc < 8:
                    emit_v_tile(8 + kc, anchor=act)
                if P is not None:
                    if kc == 0:
                        emit_av_block(P, 0, range(0, 8), anchor=act)
                    elif kc == 4:
                        emit_av_block(P, 1, range(0, 8), anchor=act)
                    elif kc == 8:
                        emit_av_block(P, 0, range(8, 16), anchor=act)
                    elif kc == 10:
                        emit_norm_a(P, 0)
                    elif kc == 12:
                        emit_av_block(P, 1, range(8, 16), anchor=act)
                    elif kc == 15:
                        emit_norm_a(P, 1)
                if P2 is not None:
                    if kc == 2:
                        emit_norm_b(P2, 0, anchor=act)
                    elif kc == 6:
                        emit_norm_b(P2, 1, anchor=act)
                        if P2[0] == 3:
                            for tt in range(4):
                                fillers.append(
                                    lambda a, t=4 * P2[1] + tt:
                                    emit_proj(t, anchor=a))
                if ui >= 2 and kc in (1, 3, 5, 7, 9, 11, 14):
                    pop_filler(act)
            P2 = P
            P = u

        # pipeline tail
        emit_av_block(P, 0, range(0, 8))
        emit_av_block(P, 0, range(8, 16))
        emit_norm_a(P, 0)
        emit_av_block(P, 1, range(0, 8))
        emit_av_block(P, 1, range(8, 16))
        emit_norm_a(P, 1)
        emit_norm_b(P2, 0)
        emit_norm_b(P2, 1)
        for tt in range(4):
            fillers.append(lambda a, t=8 + tt: emit_proj(t, anchor=a))
        pop_filler()
        pop_filler()
        emit_norm_b(P, 0)
        emit_norm_b(P, 1)
        for tt in range(4):
            fillers.append(lambda a, t=12 + tt: emit_proj(t, anchor=a))
        while fillers:
            pop_filler()


def get_program():
    if "nc" not in _prog_cache:
        _prog_cache["nc"] = _build_program()
    return _prog_cache["nc"]


def make_in_maps(inputs):
    x = np.asarray(inputs["x"], dtype=np.float32)
    Wq = np.asarray(inputs["Wq"], dtype=np.float32)
    bq = np.asarray(inputs["bq"], dtype=np.float32)
    Wk = np.asarray(inputs["Wk"], dtype=np.float32)
    bk = np.asarray(inputs["bk"], dtype=np.float32)
    Wv = np.asarray(inputs["Wv"], dtype=np.float32)
    bv = np.asarray(inputs["bv"], dtype=np.float32)
    Wp = np.asarray(inputs["Wp"], dtype=np.float32)

    ones_h = np.ones((1, 128), dtype=BF16)
    in_maps = []
    for c in range(N_CORES):
        b, half = divmod(c, 2)
        fs = slice(half * FH, half * FH + FH)
        in_maps.append({
            "xT": np.ascontiguousarray(x[b].T).astype(BF16),
            "wqT": np.ascontiguousarray(Wq[fs].T).astype(BF16),
            "wkT": np.ascontiguousarray(Wk[fs].T).astype(BF16),
            "wvT": np.ascontiguousarray(Wv[fs].T).astype(BF16),
            "bqT": np.ascontiguousarray(bq[fs].reshape(4, 128).T),
            "bkT": np.ascontiguousarray(bk[fs].reshape(4, 128).T),
            "bvs": bv[fs].astype(BF16).reshape(1, FH),
            "wpT": np.ascontiguousarray(Wp[:, fs].T).astype(BF16),
            "ones": ones_h,
        })
    return in_maps


def gather_output(results, bp):
    bp = np.asarray(bp, dtype=np.float32)
    return np.stack([
        results[2 * b]["out"].astype(np.float32)
        + results[2 * b + 1]["out"].astype(np.float32) + bp[None, :]
        for b in range(4)
    ]).astype(np.float32)


def kernel(**inputs):
    nc = get_program()
    in_maps = make_in_maps(inputs)
    res = run_bass_kernel_spmd(nc, in_maps, list(range(N_CORES))).results
    return gather_output(res, inputs["bp"])
